# revision 36
# baseline (speedup 1.0000x reference)
"""Trainium2 Bass kernel for nn_DiffKS (differentiable Karplus-Strong).

Blocked associative scan over time, two interleaved segments per core:
  1. Host (float64): cubic-spline upsampling of frame params to
     per-sample 3-tap IIR coefficients g1,g2,g3 and delays z ~ [89,317].
  2. The padded signal is split into 16 segments at chunk-aligned
     boundaries CHOSEN WHERE z IS SMALL (DP over boundary placement), so
     the per-segment state window Lmax = max_k z(boundary_k)+3 is ~half
     of the global max lag.  Each core runs TWO interleaved segments
     (slots) so the serial chain of one segment (matmul c=1 -> eviction
     -> next chunk) hides under the other's work.
  3. Per chunk of W=127 samples: 3 bf16 matmuls (c=3,2,1 history tiles)
     with NB = Lmax+1 moving columns (Lmax unit-state basis + 1
     excitation-driven column), accumulating in PSUM; DVE/ACT evicts to
     bf16 y tiles; per-group DMA streams H back to HBM.  Weight blocks
     are host-built lhsT [128, 128] (banded A_c with the in-chunk taps
     eliminated exactly via (I + A_self); row 127 carries the effective
     excitation against a ones-row of the rhs).
  4. Host composes segments with trivial matvecs
     y_seg = H[:, :Lmax] @ state + H[:, Lmax].
  bf16 rounding through the recursion gives rel err ~6e-3 (validated
  against fp64; tolerance 2e-2).
"""

import ml_dtypes
import numpy as np

import concourse.bass as bass
import concourse.mybir as mybir
import concourse.tile as tile
from concourse import bacc
from concourse.bass_utils import run_bass_kernel_spmd

F32 = mybir.dt.float32
BF16 = mybir.dt.bfloat16
BF16NP = ml_dtypes.bfloat16

N_CORES = 8
W = 127          # chunk width (samples per matmul output)
KROW = 128       # weight rows: W history samples + 1 excitation row
LEAD = 3         # history tiles before chunk 0 (3*127 >= max lag 320)
SPC = 2          # segments (slots) per core
NSEGT = N_CORES * SPC
GRP = 10         # chunks per weight/output DMA group
N_SAMP = 131072
NCH = -(-N_SAMP // W)            # 1033 chunks over the signal


# ----------------------------------------------------------------- host math
def _host_preprocess(delay_frames, raw_coeff, excitation, n_samples):
    dt = np.float64
    F = delay_frames.shape[0]
    sig = 1.0 / (1.0 + np.exp(-raw_coeff.astype(dt)))
    coeff = sig / sig.sum(-1, keepdims=True)
    t_in = np.linspace(0.0, 1.0, F).astype(dt)
    t_out = np.linspace(0.0, 1.0, n_samples).astype(dt)
    x = np.concatenate([delay_frames.astype(dt)[:, None], coeff], axis=1)
    h = t_in[1:] - t_in[:-1]
    hinv = 1.0 / h
    dx3 = 3.0 * (x[1:] - x[:-1])
    rhs_part = dx3 * (hinv * hinv)[:, None]
    diag = np.zeros(F, dt)
    diag[:-1] += hinv
    diag[1:] += hinv
    diag *= 2.0
    rhs = np.zeros_like(x)
    rhs[:-1] += rhs_part
    rhs[1:] += rhs_part
    M = np.diag(diag) + np.diag(hinv, 1) + np.diag(hinv, -1)
    k = np.linalg.solve(M, rhs)
    hc = hinv[:, None]
    a = x[:-1]
    b = k[:-1]
    two_c = (2.0 * dx3 * hc - 4.0 * k[:-1] - 2.0 * k[1:]) * hc
    three_d = (-2.0 * dx3 * hc + 3.0 * (k[:-1] + k[1:])) * hc * hc
    idx = np.clip(np.searchsorted(t_in, t_out, side="left") - 1, 0, F - 2)
    f = (t_out - t_in[idx])[:, None]
    inner = b[idx] + (0.5 * two_c[idx] + three_d[idx] * (f / 3.0)) * f
    vals = a[idx] + inner * f
    delay = vals[:, 0]
    b1 = vals[:, 1]
    b2 = vals[:, 2]
    zf = np.floor(delay)
    z = zf.astype(np.int64)
    alfa = delay - zf
    g1 = b1 * (1.0 - alfa)
    g2 = b1 * alfa + b2 * (1.0 - alfa)
    g3 = b2 * alfa
    xfull = np.zeros(n_samples, np.float64)
    nx = min(excitation.shape[0], n_samples)
    xfull[:nx] = excitation[:nx]
    return z, g1, g2, g3, xfull


def _choose_boundaries(z):
    """Pick NSEGT-1 interior chunk boundaries where z is small, trading the
    basis width NB = max boundary z + 4 against segment length CH_SEG."""
    zb = np.array([z[j * W] for j in range(1, NCH)])  # z at interior bounds

    def min_max_gap(allowed):
        # minimal L s.t. NCH splits into NSEGT gaps <= L with the NSEGT-1
        # interior boundaries drawn from `allowed` (sorted ascending)
        def feasible(L):
            # reachable boundary-k positions form a contiguous range
            # [lo, hi] of allowed entries; propagate and reconstruct back
            lo = hi = 0  # virtual position 0
            his = []
            for _ in range(NSEGT - 1):
                cand = allowed[(allowed > lo) & (allowed <= hi + L)]
                if len(cand) == 0:
                    return None
                lo, hi = int(cand[0]), int(cand[-1])
                his.append(hi)
            if NCH - hi > L:
                return None
            picks = []
            nxt = NCH
            for k in range(NSEGT - 2, -1, -1):
                cand = allowed[(allowed >= nxt - L) & (allowed < nxt)]
                cand = cand[cand <= his[k]]
                if len(cand) == 0:
                    return None
                nxt = int(cand[-1])
                picks.append(nxt)
            return picks[::-1]

        lo, hi = -(-NCH // NSEGT), NCH
        best = None
        while lo <= hi:
            mid = (lo + hi) // 2
            p = feasible(mid)
            if p is not None:
                best = (mid, p)
                hi = mid - 1
            else:
                lo = mid + 1
        return best

    best = None
    for zcap in range(int(zb.min()), int(zb.max()) + 1, 4):
        allowed = np.where(zb <= zcap)[0] + 1
        if len(allowed) < NSEGT - 1:
            continue
        r = min_max_gap(allowed)
        if r is None:
            continue
        L, picks = r
        nb = -(-(zcap + 4) // 4) * 4
        ch_seg = -(-L // GRP) * GRP
        # per chunk-pair cost model (ns): serial chain per slot vs PE vs DMA
        chain = (398 + nb) / 2.4 + 90 + nb / 0.96 + 170
        pe = 6 * (nb / 2.4 + 10)
        dma = 2 * (3 * KROW * 128 * 2 + W * nb * 2) / 320.0
        cost = ch_seg * max(chain, pe, dma)
        if best is None or cost < best[0]:
            best = (cost, nb, ch_seg, [0] + picks + [NCH])
    _, nb, ch_seg, bounds = best
    return nb, ch_seg, bounds


def _build_slot_weights(b0, ch_seg, ngrp, zp, g1p, g2p, g3p, xp):
    """lhsT blocks for chunks [b0, b0+ch_seg), packed [NG, KROW, GRP*3*128].
    Block k (c = 3-k) of chunk m at group m//GRP, col ((m%GRP)*3+k)*128."""
    s_base = b0 * W
    seg = ch_seg * W
    t = np.arange(s_base, s_base + seg)
    m_loc = (t - s_base) // W
    tl = t % W
    A = np.zeros((ch_seg, 4, W, W), np.float32)
    for j, g in ((0, g1p), (1, g2p), (2, g3p)):
        i = t - (zp[t] + 1 + j)
        c = t // W - i // W
        np.add.at(A, (m_loc, c, tl, i % W), g[t].astype(np.float32))
    A0 = A[:, 0]
    x_m = xp[s_base:s_base + seg].reshape(ch_seg, W).astype(np.float32)
    x_eff = x_m + np.einsum("mtu,mu->mt", A0, x_m)
    out = np.zeros((ngrp, KROW, GRP * 3 * KROW), BF16NP)
    for k, c in enumerate((3, 2, 1)):
        B = A[:, c] + np.matmul(A0, A[:, c])          # [m, tgt, src]
        Bt = np.ascontiguousarray(np.transpose(B, (0, 2, 1)))
        for m in range(ch_seg):
            g, off = divmod(m, GRP)
            col = (off * 3 + k) * KROW
            out[g, :W, col:col + W] = Bt[m].astype(BF16NP)
            if c == 3:
                out[g, W, col:col + W] = x_eff[m].astype(BF16NP)
    return out


# ------------------------------------------------------------- device kernel
def _build_nc(nb, ch_seg, ngrp):
    nc = bacc.Bacc(
        "TRN2", target_bir_lowering=False, debug=False, num_devices=N_CORES
    )
    wts = [
        nc.dram_tensor(f"wts{s}", [ngrp, KROW, GRP * 3 * KROW], BF16,
                       kind="ExternalInput")
        for s in range(SPC)
    ]
    init = nc.dram_tensor("init", [KROW, LEAD * nb], BF16,
                          kind="ExternalInput")
    ones = nc.dram_tensor("ones", [1, ch_seg * nb], BF16,
                          kind="ExternalInput")
    warm = nc.dram_tensor("warm", [KROW, 256], BF16, kind="ExternalInput")
    yout = nc.dram_tensor("yout", [SPC, ngrp, KROW, GRP * nb], BF16,
                          kind="ExternalOutput")
    with tile.TileContext(nc) as tc:
        with (
            tc.tile_pool(name="misc", bufs=1) as mpool,
            tc.tile_pool(name="ybuf", bufs=1) as ypool,
            tc.tile_pool(name="wpool", bufs=3) as wpool,
            tc.tile_pool(name="psum", bufs=8, space="PSUM") as ppool,
        ):
            wtile = mpool.tile([KROW, 256], BF16, tag="warm")
            nc.sync.dma_start(out=wtile[:, :], in_=warm[:, :])
            # ~5us of dummy matmuls: hold the PE busy through the HAM
            # activity window so real chunks start at 2.4 GHz
            wps = ppool.tile([W, 256], F32, tag="warmp", bufs=1)
            for i in range(8):
                nc.tensor.matmul(
                    wps[:, :], lhsT=wtile[:, 0:W], rhs=wtile[:, :],
                    start=True, stop=True,
                )
            # prime the ACT Copy table set off the critical path
            wact = mpool.tile([KROW, 256], BF16, tag="wact")
            nc.scalar.copy(wact[:, :], wtile[:, :])
            ylead = mpool.tile([KROW, LEAD * nb], BF16, tag="ylead")
            nc.sync.dma_start(out=ylead[:, :], in_=init[:, :])
            ytiles = [[None] * ngrp for _ in range(SPC)]

            def ycol(s, mm):
                if mm < 0:
                    c0 = (LEAD + mm) * nb
                    return ylead[:, c0:c0 + nb]
                g, off = divmod(mm, GRP)
                return ytiles[s][g][:, off * nb:(off + 1) * nb]

            wt = [None] * SPC
            for m in range(ch_seg):
                for s in range(SPC):
                    g, off = divmod(m, GRP)
                    if off == 0:
                        wt[s] = wpool.tile(
                            [KROW, GRP * 3 * KROW], BF16,
                            name=f"w{s}", tag=f"w{s}",
                        )
                        # slot streams ride separate DMA queues (sync
                        # HWDGE / gpsimd SWDGE) so the two group loads
                        # run in parallel instead of serializing
                        weng = nc.sync if s == 0 else nc.gpsimd
                        weng.dma_start(out=wt[s][:, :], in_=wts[s][g])
                        ytiles[s][g] = ypool.tile(
                            [KROW, GRP * nb], BF16,
                            name=f"y{s}g{g}", tag=f"y{s}g{g}",
                        )
                        nc.gpsimd.dma_start(
                            out=ytiles[s][g][W:KROW, :],
                            in_=ones[:, g * GRP * nb:(g + 1) * GRP * nb],
                        )
                    psum = ppool.tile([W, nb], F32, tag="acc", bufs=7)
                    for k, c in enumerate((3, 2, 1)):
                        col = (off * 3 + k) * KROW
                        nc.tensor.matmul(
                            psum[:, :],
                            lhsT=wt[s][:, col:col + W],
                            rhs=ycol(s, m - c),
                            start=(k == 0),
                            stop=(k == 2),
                        )
                    dst = ytiles[s][g][0:W, off * nb:(off + 1) * nb]
                    if s == 0:
                        nc.vector.tensor_copy(dst, psum[:, :])
                    else:
                        nc.scalar.copy(dst, psum[:, :])
                    if g == ngrp - 1 and off == 4:
                        # first half of the final group early: shortens
                        # the end-of-kernel store tail
                        nc.gpsimd.dma_start(
                            out=yout[s, g, :, 0:5 * nb],
                            in_=ytiles[s][g][:, 0:5 * nb],
                        )
                    if off == GRP - 1:
                        if g == ngrp - 1:
                            nc.gpsimd.dma_start(
                                out=yout[s, g, :, 5 * nb:],
                                in_=ytiles[s][g][:, 5 * nb:],
                            )
                        else:
                            nc.gpsimd.dma_start(
                                out=yout[s, g], in_=ytiles[s][g][:, :]
                            )
    nc.compile()
    return nc


_LAST_RESULT = {}


def kernel(delay_len_frames, raw_coeff_frames, excitation, n_samples):
    n = int(n_samples)
    z, g1, g2, g3, xfull = _host_preprocess(
        np.asarray(delay_len_frames), np.asarray(raw_coeff_frames),
        np.asarray(excitation), n,
    )
    assert n == N_SAMP, n
    nb, ch_seg, bounds = _choose_boundaries(z)
    ngrp = ch_seg // GRP
    lmax = nb - 1
    assert lmax <= LEAD * W
    assert int(z.min()) + 1 >= 64          # nilpotency of A_self

    npad = (NCH + ch_seg + 4) * W
    pad = npad - n
    zp = np.concatenate([z, np.full(pad, z[-1])]).astype(np.int64)
    g1p = np.concatenate([g1, np.full(pad, g1[-1])])
    g2p = np.concatenate([g2, np.full(pad, g2[-1])])
    g3p = np.concatenate([g3, np.full(pad, g3[-1])])
    xp = np.concatenate([xfull, np.zeros(pad)])

    init = np.zeros((KROW, LEAD * nb), BF16NP)
    for tt in range(LEAD):
        for r in range(W):
            j = r - (LEAD * W - lmax) + tt * W
            if 0 <= j < lmax:
                init[r, tt * nb + j] = BF16NP(1.0)
        init[W, tt * nb + nb - 1] = BF16NP(1.0)
    ones = np.zeros((1, ch_seg * nb), BF16NP)
    ones[0, nb - 1::nb] = BF16NP(1.0)
    warm = np.zeros((KROW, 256), BF16NP)

    in_maps = []
    for core in range(N_CORES):
        im = {"init": init, "ones": ones, "warm": warm}
        for s in range(SPC):
            k = core * SPC + s
            im[f"wts{s}"] = _build_slot_weights(
                bounds[k], ch_seg, ngrp, zp, g1p, g2p, g3p, xp
            )
        in_maps.append(im)

    nc = _build_nc(nb, ch_seg, ngrp)
    import os

    res = run_bass_kernel_spmd(
        nc,
        in_maps,
        core_ids=list(range(N_CORES)),
        trace=bool(os.environ.get("DIFFKS_TRACE")),
    )
    _LAST_RESULT["res"] = res

    y = np.zeros(NCH * W, np.float64)
    for k in range(NSEGT):
        core, s = divmod(k, SPC)
        H = res.results[core]["yout"][s][:, :W].astype(np.float32)
        # [ngrp, W, GRP*nb] -> [ch_seg*W, nb]
        H = H.reshape(ngrp, W, GRP, nb).transpose(0, 2, 1, 3)
        H = H.reshape(ch_seg * W, nb)
        s0 = bounds[k] * W
        ns = (bounds[k + 1] - bounds[k]) * W
        if k == 0:
            y_seg = H[:ns, lmax].astype(np.float64)
        else:
            s_k = y[s0 - lmax:s0]
            y_seg = H[:ns, :lmax].astype(np.float64) @ s_k + H[:ns, lmax]
        y[s0:s0 + ns] = y_seg
    return y[:n].astype(np.float32)


# revision 39
# speedup vs baseline: 1.1962x; 1.1962x over previous
"""Trainium2 Bass kernel for nn_DiffKS (differentiable Karplus-Strong).

Blocked associative scan over time, two interleaved segments per core:
  1. Host (float64): cubic-spline upsampling of frame params to
     per-sample 3-tap IIR coefficients g1,g2,g3 and delays z ~ [89,317].
  2. The padded signal is split into 16 segments at chunk-aligned
     boundaries CHOSEN WHERE z IS SMALL (DP over boundary placement), so
     the per-segment state window Lmax = max_k z(boundary_k)+3 is ~half
     of the global max lag.  Each core runs TWO interleaved segments
     (slots) so the serial chain of one segment (matmul c=1 -> eviction
     -> next chunk) hides under the other's work.
  3. Per chunk of W=127 samples: 3 bf16 matmuls (c=3,2,1 history tiles)
     with NB = Lmax+1 moving columns (Lmax unit-state basis + 1
     excitation-driven column), accumulating in PSUM; DVE/ACT evicts to
     bf16 y tiles; per-group DMA streams H back to HBM.  Weight blocks
     are host-built lhsT [128, 128] (banded A_c with the in-chunk taps
     eliminated exactly via (I + A_self); row 127 carries the effective
     excitation against a ones-row of the rhs).
  4. Host composes segments with trivial matvecs
     y_seg = H[:, :Lmax] @ state + H[:, Lmax].
  bf16 rounding through the recursion gives rel err ~6e-3 (validated
  against fp64; tolerance 2e-2).
"""

import ml_dtypes
import numpy as np

import concourse.bass as bass
import concourse.mybir as mybir
import concourse.tile as tile
from concourse import bacc
from concourse.bass_utils import run_bass_kernel_spmd

F32 = mybir.dt.float32
BF16 = mybir.dt.bfloat16
BF16NP = ml_dtypes.bfloat16

N_CORES = 8
W = 127          # chunk width (samples per matmul output)
KROW = 128       # weight rows: W history samples + 1 excitation row
LEAD = 3         # history tiles before chunk 0 (3*127 >= max lag 320)
SPC = 2          # segments (slots) per core
NSEGT = N_CORES * SPC
GRP = 5          # chunks per weight/output DMA group
N_SAMP = 131072
NCH = -(-N_SAMP // W)            # 1033 chunks over the signal


# ----------------------------------------------------------------- host math
def _host_preprocess(delay_frames, raw_coeff, excitation, n_samples):
    dt = np.float64
    F = delay_frames.shape[0]
    sig = 1.0 / (1.0 + np.exp(-raw_coeff.astype(dt)))
    coeff = sig / sig.sum(-1, keepdims=True)
    t_in = np.linspace(0.0, 1.0, F).astype(dt)
    t_out = np.linspace(0.0, 1.0, n_samples).astype(dt)
    x = np.concatenate([delay_frames.astype(dt)[:, None], coeff], axis=1)
    h = t_in[1:] - t_in[:-1]
    hinv = 1.0 / h
    dx3 = 3.0 * (x[1:] - x[:-1])
    rhs_part = dx3 * (hinv * hinv)[:, None]
    diag = np.zeros(F, dt)
    diag[:-1] += hinv
    diag[1:] += hinv
    diag *= 2.0
    rhs = np.zeros_like(x)
    rhs[:-1] += rhs_part
    rhs[1:] += rhs_part
    M = np.diag(diag) + np.diag(hinv, 1) + np.diag(hinv, -1)
    k = np.linalg.solve(M, rhs)
    hc = hinv[:, None]
    a = x[:-1]
    b = k[:-1]
    two_c = (2.0 * dx3 * hc - 4.0 * k[:-1] - 2.0 * k[1:]) * hc
    three_d = (-2.0 * dx3 * hc + 3.0 * (k[:-1] + k[1:])) * hc * hc
    idx = np.clip(np.searchsorted(t_in, t_out, side="left") - 1, 0, F - 2)
    f = (t_out - t_in[idx])[:, None]
    inner = b[idx] + (0.5 * two_c[idx] + three_d[idx] * (f / 3.0)) * f
    vals = a[idx] + inner * f
    delay = vals[:, 0]
    b1 = vals[:, 1]
    b2 = vals[:, 2]
    zf = np.floor(delay)
    z = zf.astype(np.int64)
    alfa = delay - zf
    g1 = b1 * (1.0 - alfa)
    g2 = b1 * alfa + b2 * (1.0 - alfa)
    g3 = b2 * alfa
    xfull = np.zeros(n_samples, np.float64)
    nx = min(excitation.shape[0], n_samples)
    xfull[:nx] = excitation[:nx]
    return z, g1, g2, g3, xfull


def _choose_boundaries(z):
    """Pick NSEGT-1 interior chunk boundaries where z is small, trading the
    basis width NB = max boundary z + 4 against segment length CH_SEG."""
    zb = np.array([z[j * W] for j in range(1, NCH)])  # z at interior bounds

    def min_max_gap(allowed):
        # minimal L s.t. NCH splits into NSEGT gaps <= L with the NSEGT-1
        # interior boundaries drawn from `allowed` (sorted ascending)
        def feasible(L):
            # reachable boundary-k positions form a contiguous range
            # [lo, hi] of allowed entries; propagate and reconstruct back
            lo = hi = 0  # virtual position 0
            his = []
            for _ in range(NSEGT - 1):
                cand = allowed[(allowed > lo) & (allowed <= hi + L)]
                if len(cand) == 0:
                    return None
                lo, hi = int(cand[0]), int(cand[-1])
                his.append(hi)
            if NCH - hi > L:
                return None
            picks = []
            nxt = NCH
            for k in range(NSEGT - 2, -1, -1):
                cand = allowed[(allowed >= nxt - L) & (allowed < nxt)]
                cand = cand[cand <= his[k]]
                if len(cand) == 0:
                    return None
                nxt = int(cand[-1])
                picks.append(nxt)
            return picks[::-1]

        lo, hi = -(-NCH // NSEGT), NCH
        best = None
        while lo <= hi:
            mid = (lo + hi) // 2
            p = feasible(mid)
            if p is not None:
                best = (mid, p)
                hi = mid - 1
            else:
                lo = mid + 1
        return best

    best = None
    for zcap in range(int(zb.min()), int(zb.max()) + 1, 4):
        allowed = np.where(zb <= zcap)[0] + 1
        if len(allowed) < NSEGT - 1:
            continue
        r = min_max_gap(allowed)
        if r is None:
            continue
        L, picks = r
        nb = -(-(zcap + 4) // 4) * 4
        ch_seg = -(-L // GRP) * GRP
        # per chunk-pair cost model (ns): serial chain per slot vs PE vs DMA
        chain = (398 + nb) / 2.4 + 90 + nb / 0.96 + 170
        pe = 6 * (nb / 2.4 + 10)
        dma = 2 * (3 * KROW * 128 * 2 + W * nb * 2) / 320.0
        cost = ch_seg * max(chain, pe, dma)
        if best is None or cost < best[0]:
            best = (cost, nb, ch_seg, [0] + picks + [NCH])
    _, nb, ch_seg, bounds = best
    return nb, ch_seg, bounds


def _build_slot_weights(b0, ch_seg, ngrp, zp, g1p, g2p, g3p, xp):
    """lhsT blocks for chunks [b0, b0+ch_seg), packed [NG, KROW, GRP*3*128].
    Block k (c = 3-k) of chunk m at group m//GRP, col ((m%GRP)*3+k)*128."""
    s_base = b0 * W
    seg = ch_seg * W
    t = np.arange(s_base, s_base + seg)
    m_loc = (t - s_base) // W
    tl = t % W
    A = np.zeros((ch_seg, 4, W, W), np.float32)
    for j, g in ((0, g1p), (1, g2p), (2, g3p)):
        i = t - (zp[t] + 1 + j)
        c = t // W - i // W
        np.add.at(A, (m_loc, c, tl, i % W), g[t].astype(np.float32))
    A0 = A[:, 0]
    x_m = xp[s_base:s_base + seg].reshape(ch_seg, W).astype(np.float32)
    x_eff = x_m + np.einsum("mtu,mu->mt", A0, x_m)
    out = np.zeros((ngrp, KROW, GRP * 3 * KROW), BF16NP)
    for k, c in enumerate((3, 2, 1)):
        B = A[:, c] + np.matmul(A0, A[:, c])          # [m, tgt, src]
        Bt = np.ascontiguousarray(np.transpose(B, (0, 2, 1)))
        for m in range(ch_seg):
            g, off = divmod(m, GRP)
            col = (off * 3 + k) * KROW
            out[g, :W, col:col + W] = Bt[m].astype(BF16NP)
            if c == 3:
                out[g, W, col:col + W] = x_eff[m].astype(BF16NP)
    return out


# ------------------------------------------------------------- device kernel
def _build_nc(nb, ch_seg, ngrp):
    nc = bacc.Bacc(
        "TRN2", target_bir_lowering=False, debug=False, num_devices=N_CORES
    )
    wts = [
        nc.dram_tensor(f"wts{s}", [ngrp, KROW, GRP * 3 * KROW], BF16,
                       kind="ExternalInput")
        for s in range(SPC)
    ]
    init = nc.dram_tensor("init", [KROW, LEAD * nb], BF16,
                          kind="ExternalInput")
    ones = nc.dram_tensor("ones", [1, ch_seg * nb], BF16,
                          kind="ExternalInput")
    warm = nc.dram_tensor("warm", [KROW, 256], BF16, kind="ExternalInput")
    yout = nc.dram_tensor("yout", [SPC, ngrp, KROW, GRP * nb], BF16,
                          kind="ExternalOutput")
    with tile.TileContext(nc) as tc:
        with (
            tc.tile_pool(name="misc", bufs=1) as mpool,
            tc.tile_pool(name="ybuf", bufs=1) as ypool,
            tc.tile_pool(name="wpool", bufs=3) as wpool,
            tc.tile_pool(name="psum", bufs=8, space="PSUM") as ppool,
        ):
            wtile = mpool.tile([KROW, 256], BF16, tag="warm")
            nc.sync.dma_start(out=wtile[:, :], in_=warm[:, :])
            # ~5us of dummy matmuls: hold the PE busy through the HAM
            # activity window so real chunks start at 2.4 GHz
            wps = ppool.tile([W, 256], F32, tag="warmp", bufs=1)
            for i in range(8):
                nc.tensor.matmul(
                    wps[:, :], lhsT=wtile[:, 0:W], rhs=wtile[:, :],
                    start=True, stop=True,
                )
            # prime the ACT Copy table set off the critical path
            wact = mpool.tile([KROW, 256], BF16, tag="wact")
            nc.scalar.copy(wact[:, :], wtile[:, :])
            ylead = mpool.tile([KROW, LEAD * nb], BF16, tag="ylead")
            nc.sync.dma_start(out=ylead[:, :], in_=init[:, :])
            ytiles = [[None] * ngrp for _ in range(SPC)]

            def ycol(s, mm):
                if mm < 0:
                    c0 = (LEAD + mm) * nb
                    return ylead[:, c0:c0 + nb]
                g, off = divmod(mm, GRP)
                return ytiles[s][g][:, off * nb:(off + 1) * nb]

            wt = [None] * SPC
            for m in range(ch_seg):
                for s in range(SPC):
                    g, off = divmod(m, GRP)
                    if off == 0:
                        wt[s] = wpool.tile(
                            [KROW, GRP * 3 * KROW], BF16,
                            name=f"w{s}", tag=f"w{s}",
                        )
                        nc.sync.dma_start(out=wt[s][:, :], in_=wts[s][g])
                        ytiles[s][g] = ypool.tile(
                            [KROW, GRP * nb], BF16,
                            name=f"y{s}g{g}", tag=f"y{s}g{g}",
                        )
                        nc.gpsimd.dma_start(
                            out=ytiles[s][g][W:KROW, :],
                            in_=ones[:, g * GRP * nb:(g + 1) * GRP * nb],
                        )
                    psum = ppool.tile([W, nb], F32, tag="acc", bufs=7)
                    for k, c in enumerate((3, 2, 1)):
                        col = (off * 3 + k) * KROW
                        nc.tensor.matmul(
                            psum[:, :],
                            lhsT=wt[s][:, col:col + W],
                            rhs=ycol(s, m - c),
                            start=(k == 0),
                            stop=(k == 2),
                        )
                    dst = ytiles[s][g][0:W, off * nb:(off + 1) * nb]
                    if s == 0:
                        nc.vector.tensor_copy(dst, psum[:, :])
                    else:
                        nc.scalar.copy(dst, psum[:, :])
                    if off == GRP - 1:
                        nc.gpsimd.dma_start(
                            out=yout[s, g], in_=ytiles[s][g][:, :]
                        )
    nc.compile()
    return nc


_LAST_RESULT = {}


def kernel(delay_len_frames, raw_coeff_frames, excitation, n_samples):
    n = int(n_samples)
    z, g1, g2, g3, xfull = _host_preprocess(
        np.asarray(delay_len_frames), np.asarray(raw_coeff_frames),
        np.asarray(excitation), n,
    )
    assert n == N_SAMP, n
    nb, ch_seg, bounds = _choose_boundaries(z)
    ngrp = ch_seg // GRP
    lmax = nb - 1
    assert lmax <= LEAD * W
    assert int(z.min()) + 1 >= 64          # nilpotency of A_self

    npad = (NCH + ch_seg + 4) * W
    pad = npad - n
    zp = np.concatenate([z, np.full(pad, z[-1])]).astype(np.int64)
    g1p = np.concatenate([g1, np.full(pad, g1[-1])])
    g2p = np.concatenate([g2, np.full(pad, g2[-1])])
    g3p = np.concatenate([g3, np.full(pad, g3[-1])])
    xp = np.concatenate([xfull, np.zeros(pad)])

    init = np.zeros((KROW, LEAD * nb), BF16NP)
    for tt in range(LEAD):
        for r in range(W):
            j = r - (LEAD * W - lmax) + tt * W
            if 0 <= j < lmax:
                init[r, tt * nb + j] = BF16NP(1.0)
        init[W, tt * nb + nb - 1] = BF16NP(1.0)
    ones = np.zeros((1, ch_seg * nb), BF16NP)
    ones[0, nb - 1::nb] = BF16NP(1.0)
    warm = np.zeros((KROW, 256), BF16NP)

    in_maps = []
    for core in range(N_CORES):
        im = {"init": init, "ones": ones, "warm": warm}
        for s in range(SPC):
            k = core * SPC + s
            im[f"wts{s}"] = _build_slot_weights(
                bounds[k], ch_seg, ngrp, zp, g1p, g2p, g3p, xp
            )
        in_maps.append(im)

    nc = _build_nc(nb, ch_seg, ngrp)
    import os

    res = run_bass_kernel_spmd(
        nc,
        in_maps,
        core_ids=list(range(N_CORES)),
        trace=bool(os.environ.get("DIFFKS_TRACE")),
    )
    _LAST_RESULT["res"] = res

    y = np.zeros(NCH * W, np.float64)
    for k in range(NSEGT):
        core, s = divmod(k, SPC)
        H = res.results[core]["yout"][s][:, :W].astype(np.float32)
        # [ngrp, W, GRP*nb] -> [ch_seg*W, nb]
        H = H.reshape(ngrp, W, GRP, nb).transpose(0, 2, 1, 3)
        H = H.reshape(ch_seg * W, nb)
        s0 = bounds[k] * W
        ns = (bounds[k + 1] - bounds[k]) * W
        if k == 0:
            y_seg = H[:ns, lmax].astype(np.float64)
        else:
            s_k = y[s0 - lmax:s0]
            y_seg = H[:ns, :lmax].astype(np.float64) @ s_k + H[:ns, lmax]
        y[s0:s0 + ns] = y_seg
    return y[:n].astype(np.float32)


# revision 43
# speedup vs baseline: 1.3236x; 1.1065x over previous
"""Trainium2 Bass kernel for nn_DiffKS (differentiable Karplus-Strong).

Blocked associative scan over time, two interleaved segments per core:
  1. Host (float64): cubic-spline upsampling of frame params to
     per-sample 3-tap IIR coefficients g1,g2,g3 and delays z ~ [89,317].
  2. The padded signal is split into 16 segments at chunk-aligned
     boundaries CHOSEN WHERE z IS SMALL (DP over boundary placement), so
     the per-segment state window Lmax = max_k z(boundary_k)+3 is ~half
     of the global max lag.  Each core runs TWO interleaved segments
     (slots) so the serial chain of one segment (matmul c=1 -> eviction
     -> next chunk) hides under the other's work.
  3. Per chunk of W=127 samples: 3 bf16 matmuls (c=3,2,1 history tiles)
     with NB = Lmax+1 moving columns (Lmax unit-state basis + 1
     excitation-driven column), accumulating in PSUM; DVE/ACT evicts to
     bf16 y tiles; per-group DMA streams H back to HBM.  Weight blocks
     are host-built lhsT [128, 128] (banded A_c with the in-chunk taps
     eliminated exactly via (I + A_self); row 127 carries the effective
     excitation against a ones-row of the rhs).
  4. Host composes segments with trivial matvecs
     y_seg = H[:, :Lmax] @ state + H[:, Lmax].
  bf16 rounding through the recursion gives rel err ~6e-3 (validated
  against fp64; tolerance 2e-2).
"""

import ml_dtypes
import numpy as np

import concourse.bass as bass
import concourse.mybir as mybir
import concourse.tile as tile
from concourse import bacc
from concourse.bass_utils import run_bass_kernel_spmd

F32 = mybir.dt.float32
BF16 = mybir.dt.bfloat16
BF16NP = ml_dtypes.bfloat16

N_CORES = 8
W = 127          # chunk width (samples per matmul output)
KROW = 128       # weight rows: W history samples + 1 excitation row
LEAD = 3         # history tiles before chunk 0 (3*127 >= max lag 320)
SPC = 2          # segments (slots) per core
NSEGT = N_CORES * SPC
GRP = 10         # chunks per weight/output DMA group
N_SAMP = 131072
NCH = -(-N_SAMP // W)            # 1033 chunks over the signal


# ----------------------------------------------------------------- host math
def _host_preprocess(delay_frames, raw_coeff, excitation, n_samples):
    dt = np.float64
    F = delay_frames.shape[0]
    sig = 1.0 / (1.0 + np.exp(-raw_coeff.astype(dt)))
    coeff = sig / sig.sum(-1, keepdims=True)
    t_in = np.linspace(0.0, 1.0, F).astype(dt)
    t_out = np.linspace(0.0, 1.0, n_samples).astype(dt)
    x = np.concatenate([delay_frames.astype(dt)[:, None], coeff], axis=1)
    h = t_in[1:] - t_in[:-1]
    hinv = 1.0 / h
    dx3 = 3.0 * (x[1:] - x[:-1])
    rhs_part = dx3 * (hinv * hinv)[:, None]
    diag = np.zeros(F, dt)
    diag[:-1] += hinv
    diag[1:] += hinv
    diag *= 2.0
    rhs = np.zeros_like(x)
    rhs[:-1] += rhs_part
    rhs[1:] += rhs_part
    M = np.diag(diag) + np.diag(hinv, 1) + np.diag(hinv, -1)
    k = np.linalg.solve(M, rhs)
    hc = hinv[:, None]
    a = x[:-1]
    b = k[:-1]
    two_c = (2.0 * dx3 * hc - 4.0 * k[:-1] - 2.0 * k[1:]) * hc
    three_d = (-2.0 * dx3 * hc + 3.0 * (k[:-1] + k[1:])) * hc * hc
    idx = np.clip(np.searchsorted(t_in, t_out, side="left") - 1, 0, F - 2)
    f = (t_out - t_in[idx])[:, None]
    inner = b[idx] + (0.5 * two_c[idx] + three_d[idx] * (f / 3.0)) * f
    vals = a[idx] + inner * f
    delay = vals[:, 0]
    b1 = vals[:, 1]
    b2 = vals[:, 2]
    zf = np.floor(delay)
    z = zf.astype(np.int64)
    alfa = delay - zf
    g1 = b1 * (1.0 - alfa)
    g2 = b1 * alfa + b2 * (1.0 - alfa)
    g3 = b2 * alfa
    xfull = np.zeros(n_samples, np.float64)
    nx = min(excitation.shape[0], n_samples)
    xfull[:nx] = excitation[:nx]
    return z, g1, g2, g3, xfull


def _choose_boundaries(z):
    """Pick NSEGT-1 interior chunk boundaries where z is small, trading the
    basis width NB = max boundary z + 4 against segment length CH_SEG."""
    zb = np.array([z[j * W] for j in range(1, NCH)])  # z at interior bounds

    def min_max_gap(allowed):
        # minimal L s.t. NCH splits into NSEGT gaps <= L with the NSEGT-1
        # interior boundaries drawn from `allowed` (sorted ascending)
        def feasible(L):
            # reachable boundary-k positions form a contiguous range
            # [lo, hi] of allowed entries; propagate and reconstruct back
            lo = hi = 0  # virtual position 0
            his = []
            for _ in range(NSEGT - 1):
                cand = allowed[(allowed > lo) & (allowed <= hi + L)]
                if len(cand) == 0:
                    return None
                lo, hi = int(cand[0]), int(cand[-1])
                his.append(hi)
            if NCH - hi > L:
                return None
            picks = []
            nxt = NCH
            for k in range(NSEGT - 2, -1, -1):
                cand = allowed[(allowed >= nxt - L) & (allowed < nxt)]
                cand = cand[cand <= his[k]]
                if len(cand) == 0:
                    return None
                nxt = int(cand[-1])
                picks.append(nxt)
            return picks[::-1]

        lo, hi = -(-NCH // NSEGT), NCH
        best = None
        while lo <= hi:
            mid = (lo + hi) // 2
            p = feasible(mid)
            if p is not None:
                best = (mid, p)
                hi = mid - 1
            else:
                lo = mid + 1
        return best

    best = None
    for zcap in range(int(zb.min()), int(zb.max()) + 1, 4):
        allowed = np.where(zb <= zcap)[0] + 1
        if len(allowed) < NSEGT - 1:
            continue
        r = min_max_gap(allowed)
        if r is None:
            continue
        L, picks = r
        nb = -(-(zcap + 4) // 4) * 4
        ch_seg = -(-L // GRP) * GRP
        # per chunk-pair cost model (ns): serial chain per slot vs PE vs DMA
        chain = (398 + nb) / 2.4 + 90 + nb / 0.96 + 170
        pe = 6 * (nb / 2.4 + 10)
        dma = 2 * (3 * KROW * 128 * 2 + W * nb * 2) / 320.0
        cost = ch_seg * max(chain, pe, dma)
        if best is None or cost < best[0]:
            best = (cost, nb, ch_seg, [0] + picks + [NCH])
    _, nb, ch_seg, bounds = best
    return nb, ch_seg, bounds


def _build_slot_weights(b0, ch_seg, ngrp, zp, g1p, g2p, g3p, xp):
    """lhsT blocks for chunks [b0, b0+ch_seg), packed [NG, KROW, GRP*3*128].
    Block k (c = 3-k) of chunk m at group m//GRP, col ((m%GRP)*3+k)*128."""
    s_base = b0 * W
    seg = ch_seg * W
    t = np.arange(s_base, s_base + seg)
    m_loc = (t - s_base) // W
    tl = t % W
    A = np.zeros((ch_seg, 4, W, W), np.float32)
    for j, g in ((0, g1p), (1, g2p), (2, g3p)):
        i = t - (zp[t] + 1 + j)
        c = t // W - i // W
        np.add.at(A, (m_loc, c, tl, i % W), g[t].astype(np.float32))
    A0 = A[:, 0]
    x_m = xp[s_base:s_base + seg].reshape(ch_seg, W).astype(np.float32)
    x_eff = x_m + np.einsum("mtu,mu->mt", A0, x_m)
    out = np.zeros((ngrp, KROW, GRP * 3 * KROW), BF16NP)
    for k, c in enumerate((3, 2, 1)):
        B = A[:, c] + np.matmul(A0, A[:, c])          # [m, tgt, src]
        Bt = np.ascontiguousarray(np.transpose(B, (0, 2, 1)))
        for m in range(ch_seg):
            g, off = divmod(m, GRP)
            col = (off * 3 + k) * KROW
            out[g, :W, col:col + W] = Bt[m].astype(BF16NP)
            if c == 3:
                out[g, W, col:col + W] = x_eff[m].astype(BF16NP)
    return out


# ------------------------------------------------------------- device kernel
def _build_nc(nb, ch_seg, ngrp):
    nc = bacc.Bacc(
        "TRN2", target_bir_lowering=False, debug=False, num_devices=N_CORES
    )
    wts = [
        nc.dram_tensor(f"wts{s}", [ngrp, KROW, GRP * 3 * KROW], BF16,
                       kind="ExternalInput")
        for s in range(SPC)
    ]
    init = nc.dram_tensor("init", [KROW, LEAD * nb], BF16,
                          kind="ExternalInput")
    ones = nc.dram_tensor("ones", [1, ch_seg * nb], BF16,
                          kind="ExternalInput")
    warm = nc.dram_tensor("warm", [KROW, 256], BF16, kind="ExternalInput")
    yout = nc.dram_tensor("yout", [SPC, ngrp, KROW, GRP * nb], BF16,
                          kind="ExternalOutput")
    with tile.TileContext(nc) as tc:
        with (
            tc.tile_pool(name="misc", bufs=1) as mpool,
            tc.tile_pool(name="ybuf", bufs=1) as ypool,
            tc.tile_pool(name="wpool", bufs=3) as wpool,
            tc.tile_pool(name="psum", bufs=8, space="PSUM") as ppool,
        ):
            wtile = mpool.tile([KROW, 256], BF16, tag="warm")
            nc.sync.dma_start(out=wtile[:, :], in_=warm[:, :])
            # ~5us of dummy matmuls: hold the PE busy through the HAM
            # activity window so real chunks start at 2.4 GHz
            wps = ppool.tile([W, 256], F32, tag="warmp", bufs=1)
            for i in range(9):
                nc.tensor.matmul(
                    wps[:, :], lhsT=wtile[:, 0:W], rhs=wtile[:, :],
                    start=True, stop=True,
                )
            # prime the ACT Copy table set off the critical path
            wact = mpool.tile([KROW, 256], BF16, tag="wact")
            nc.scalar.copy(wact[:, :], wtile[:, :])
            ylead = mpool.tile([KROW, LEAD * nb], BF16, tag="ylead")
            nc.sync.dma_start(out=ylead[:, :], in_=init[:, :])
            ytiles = [[None] * ngrp for _ in range(SPC)]

            def ycol(s, mm):
                if mm < 0:
                    c0 = (LEAD + mm) * nb
                    return ylead[:, c0:c0 + nb]
                g, off = divmod(mm, GRP)
                return ytiles[s][g][:, off * nb:(off + 1) * nb]

            # group 0 lands as interleaved 5-chunk pieces per slot so the
            # first pairs start ~4us earlier than two full-group waits
            wt = [None] * SPC
            wt0 = []
            c0 = 5 * 3 * KROW
            for s in range(SPC):
                t = wpool.tile(
                    [KROW, GRP * 3 * KROW], BF16, name=f"w{s}", tag=f"w{s}"
                )
                nc.sync.dma_start(out=t[:, 0:c0], in_=wts[s][0][:, 0:c0])
                wt0.append(t)
            for s in range(SPC):
                nc.sync.dma_start(out=wt0[s][:, c0:], in_=wts[s][0][:, c0:])
            for m in range(ch_seg):
                for s in range(SPC):
                    g, off = divmod(m, GRP)
                    if off == 0:
                        if g == 0:
                            wt[s] = wt0[s]
                        else:
                            wt[s] = wpool.tile(
                                [KROW, GRP * 3 * KROW], BF16,
                                name=f"w{s}", tag=f"w{s}",
                            )
                            nc.sync.dma_start(
                                out=wt[s][:, :], in_=wts[s][g]
                            )
                        ytiles[s][g] = ypool.tile(
                            [KROW, GRP * nb], BF16,
                            name=f"y{s}g{g}", tag=f"y{s}g{g}",
                        )
                        nc.gpsimd.dma_start(
                            out=ytiles[s][g][W:KROW, :],
                            in_=ones[:, g * GRP * nb:(g + 1) * GRP * nb],
                        )
                    psum = ppool.tile([W, nb], F32, tag="acc", bufs=7)
                    for k, c in enumerate((3, 2, 1)):
                        col = (off * 3 + k) * KROW
                        nc.tensor.matmul(
                            psum[:, :],
                            lhsT=wt[s][:, col:col + W],
                            rhs=ycol(s, m - c),
                            start=(k == 0),
                            stop=(k == 2),
                        )
                    dst = ytiles[s][g][0:W, off * nb:(off + 1) * nb]
                    if s == 0:
                        nc.vector.tensor_copy(dst, psum[:, :])
                    else:
                        nc.scalar.copy(dst, psum[:, :])
                    if g == ngrp - 1 and off == 4:
                        # first half of the final group early: shortens
                        # the end-of-kernel store tail
                        nc.gpsimd.dma_start(
                            out=yout[s, g, :, 0:5 * nb],
                            in_=ytiles[s][g][:, 0:5 * nb],
                        )
                    if off == GRP - 1:
                        if g == ngrp - 1:
                            nc.gpsimd.dma_start(
                                out=yout[s, g, :, 5 * nb:],
                                in_=ytiles[s][g][:, 5 * nb:],
                            )
                        else:
                            nc.gpsimd.dma_start(
                                out=yout[s, g], in_=ytiles[s][g][:, :]
                            )
    nc.compile()
    return nc


_LAST_RESULT = {}


def kernel(delay_len_frames, raw_coeff_frames, excitation, n_samples):
    n = int(n_samples)
    z, g1, g2, g3, xfull = _host_preprocess(
        np.asarray(delay_len_frames), np.asarray(raw_coeff_frames),
        np.asarray(excitation), n,
    )
    assert n == N_SAMP, n
    nb, ch_seg, bounds = _choose_boundaries(z)
    ngrp = ch_seg // GRP
    lmax = nb - 1
    assert lmax <= LEAD * W
    assert int(z.min()) + 1 >= 64          # nilpotency of A_self

    npad = (NCH + ch_seg + 4) * W
    pad = npad - n
    zp = np.concatenate([z, np.full(pad, z[-1])]).astype(np.int64)
    g1p = np.concatenate([g1, np.full(pad, g1[-1])])
    g2p = np.concatenate([g2, np.full(pad, g2[-1])])
    g3p = np.concatenate([g3, np.full(pad, g3[-1])])
    xp = np.concatenate([xfull, np.zeros(pad)])

    init = np.zeros((KROW, LEAD * nb), BF16NP)
    for tt in range(LEAD):
        for r in range(W):
            j = r - (LEAD * W - lmax) + tt * W
            if 0 <= j < lmax:
                init[r, tt * nb + j] = BF16NP(1.0)
        init[W, tt * nb + nb - 1] = BF16NP(1.0)
    ones = np.zeros((1, ch_seg * nb), BF16NP)
    ones[0, nb - 1::nb] = BF16NP(1.0)
    warm = np.zeros((KROW, 256), BF16NP)

    in_maps = []
    for core in range(N_CORES):
        im = {"init": init, "ones": ones, "warm": warm}
        for s in range(SPC):
            k = core * SPC + s
            im[f"wts{s}"] = _build_slot_weights(
                bounds[k], ch_seg, ngrp, zp, g1p, g2p, g3p, xp
            )
        in_maps.append(im)

    nc = _build_nc(nb, ch_seg, ngrp)
    import os

    res = run_bass_kernel_spmd(
        nc,
        in_maps,
        core_ids=list(range(N_CORES)),
        trace=bool(os.environ.get("DIFFKS_TRACE")),
    )
    _LAST_RESULT["res"] = res

    y = np.zeros(NCH * W, np.float64)
    for k in range(NSEGT):
        core, s = divmod(k, SPC)
        H = res.results[core]["yout"][s][:, :W].astype(np.float32)
        # [ngrp, W, GRP*nb] -> [ch_seg*W, nb]
        H = H.reshape(ngrp, W, GRP, nb).transpose(0, 2, 1, 3)
        H = H.reshape(ch_seg * W, nb)
        s0 = bounds[k] * W
        ns = (bounds[k + 1] - bounds[k]) * W
        if k == 0:
            y_seg = H[:ns, lmax].astype(np.float64)
        else:
            s_k = y[s0 - lmax:s0]
            y_seg = H[:ns, :lmax].astype(np.float64) @ s_k + H[:ns, lmax]
        y[s0:s0 + ns] = y_seg
    return y[:n].astype(np.float32)


# revision 44
# speedup vs baseline: 1.3437x; 1.0151x over previous
"""Trainium2 Bass kernel for nn_DiffKS (differentiable Karplus-Strong).

Blocked associative scan over time, two interleaved segments per core:
  1. Host (float64): cubic-spline upsampling of frame params to
     per-sample 3-tap IIR coefficients g1,g2,g3 and delays z ~ [89,317].
  2. The padded signal is split into 16 segments at chunk-aligned
     boundaries CHOSEN WHERE z IS SMALL (DP over boundary placement), so
     the per-segment state window Lmax = max_k z(boundary_k)+3 is ~half
     of the global max lag.  Each core runs TWO interleaved segments
     (slots) so the serial chain of one segment (matmul c=1 -> eviction
     -> next chunk) hides under the other's work.
  3. Per chunk of W=127 samples: 3 bf16 matmuls (c=3,2,1 history tiles)
     with NB = Lmax+1 moving columns (Lmax unit-state basis + 1
     excitation-driven column), accumulating in PSUM; DVE/ACT evicts to
     bf16 y tiles; per-group DMA streams H back to HBM.  Weight blocks
     are host-built lhsT [128, 128] (banded A_c with the in-chunk taps
     eliminated exactly via (I + A_self); row 127 carries the effective
     excitation against a ones-row of the rhs).
  4. Host composes segments with trivial matvecs
     y_seg = H[:, :Lmax] @ state + H[:, Lmax].
  bf16 rounding through the recursion gives rel err ~6e-3 (validated
  against fp64; tolerance 2e-2).
"""

import ml_dtypes
import numpy as np

import concourse.bass as bass
import concourse.mybir as mybir
import concourse.tile as tile
from concourse import bacc
from concourse.bass_utils import run_bass_kernel_spmd

F32 = mybir.dt.float32
BF16 = mybir.dt.bfloat16
BF16NP = ml_dtypes.bfloat16

N_CORES = 8
W = 127          # chunk width (samples per matmul output)
KROW = 128       # weight rows: W history samples + 1 excitation row
LEAD = 3         # history tiles before chunk 0 (3*127 >= max lag 320)
SPC = 2          # segments (slots) per core
NSEGT = N_CORES * SPC
GRP = 10         # chunks per weight/output DMA group
N_SAMP = 131072
NCH = -(-N_SAMP // W)            # 1033 chunks over the signal


# ----------------------------------------------------------------- host math
def _host_preprocess(delay_frames, raw_coeff, excitation, n_samples):
    dt = np.float64
    F = delay_frames.shape[0]
    sig = 1.0 / (1.0 + np.exp(-raw_coeff.astype(dt)))
    coeff = sig / sig.sum(-1, keepdims=True)
    t_in = np.linspace(0.0, 1.0, F).astype(dt)
    t_out = np.linspace(0.0, 1.0, n_samples).astype(dt)
    x = np.concatenate([delay_frames.astype(dt)[:, None], coeff], axis=1)
    h = t_in[1:] - t_in[:-1]
    hinv = 1.0 / h
    dx3 = 3.0 * (x[1:] - x[:-1])
    rhs_part = dx3 * (hinv * hinv)[:, None]
    diag = np.zeros(F, dt)
    diag[:-1] += hinv
    diag[1:] += hinv
    diag *= 2.0
    rhs = np.zeros_like(x)
    rhs[:-1] += rhs_part
    rhs[1:] += rhs_part
    M = np.diag(diag) + np.diag(hinv, 1) + np.diag(hinv, -1)
    k = np.linalg.solve(M, rhs)
    hc = hinv[:, None]
    a = x[:-1]
    b = k[:-1]
    two_c = (2.0 * dx3 * hc - 4.0 * k[:-1] - 2.0 * k[1:]) * hc
    three_d = (-2.0 * dx3 * hc + 3.0 * (k[:-1] + k[1:])) * hc * hc
    idx = np.clip(np.searchsorted(t_in, t_out, side="left") - 1, 0, F - 2)
    f = (t_out - t_in[idx])[:, None]
    inner = b[idx] + (0.5 * two_c[idx] + three_d[idx] * (f / 3.0)) * f
    vals = a[idx] + inner * f
    delay = vals[:, 0]
    b1 = vals[:, 1]
    b2 = vals[:, 2]
    zf = np.floor(delay)
    z = zf.astype(np.int64)
    alfa = delay - zf
    g1 = b1 * (1.0 - alfa)
    g2 = b1 * alfa + b2 * (1.0 - alfa)
    g3 = b2 * alfa
    xfull = np.zeros(n_samples, np.float64)
    nx = min(excitation.shape[0], n_samples)
    xfull[:nx] = excitation[:nx]
    return z, g1, g2, g3, xfull


def _choose_boundaries(z):
    """Pick NSEGT-1 interior chunk boundaries where z is small, trading the
    basis width NB = max boundary z + 4 against segment length CH_SEG."""
    zb = np.array([z[j * W] for j in range(1, NCH)])  # z at interior bounds

    def min_max_gap(allowed):
        # minimal L s.t. NCH splits into NSEGT gaps <= L with the NSEGT-1
        # interior boundaries drawn from `allowed` (sorted ascending)
        def feasible(L):
            # reachable boundary-k positions form a contiguous range
            # [lo, hi] of allowed entries; propagate and reconstruct back
            lo = hi = 0  # virtual position 0
            his = []
            for _ in range(NSEGT - 1):
                cand = allowed[(allowed > lo) & (allowed <= hi + L)]
                if len(cand) == 0:
                    return None
                lo, hi = int(cand[0]), int(cand[-1])
                his.append(hi)
            if NCH - hi > L:
                return None
            picks = []
            nxt = NCH
            for k in range(NSEGT - 2, -1, -1):
                cand = allowed[(allowed >= nxt - L) & (allowed < nxt)]
                cand = cand[cand <= his[k]]
                if len(cand) == 0:
                    return None
                nxt = int(cand[-1])
                picks.append(nxt)
            return picks[::-1]

        lo, hi = -(-NCH // NSEGT), NCH
        best = None
        while lo <= hi:
            mid = (lo + hi) // 2
            p = feasible(mid)
            if p is not None:
                best = (mid, p)
                hi = mid - 1
            else:
                lo = mid + 1
        return best

    best = None
    for zcap in range(int(zb.min()), int(zb.max()) + 1, 4):
        allowed = np.where(zb <= zcap)[0] + 1
        if len(allowed) < NSEGT - 1:
            continue
        r = min_max_gap(allowed)
        if r is None:
            continue
        L, picks = r
        nb = -(-(zcap + 4) // 4) * 4
        ch_seg = -(-L // GRP) * GRP
        # per chunk-pair cost model (ns): serial chain per slot vs PE vs DMA
        chain = (398 + nb) / 2.4 + 90 + nb / 0.96 + 170
        pe = 6 * (nb / 2.4 + 10)
        dma = 2 * (3 * KROW * 128 * 2 + W * nb * 2) / 320.0
        cost = ch_seg * max(chain, pe, dma)
        if best is None or cost < best[0]:
            best = (cost, nb, ch_seg, [0] + picks + [NCH])
    _, nb, ch_seg, bounds = best
    return nb, ch_seg, bounds


def _build_slot_weights(b0, ch_seg, ngrp, zp, g1p, g2p, g3p, xp):
    """lhsT blocks for chunks [b0, b0+ch_seg), packed [NG, KROW, GRP*3*128].
    Block k (c = 3-k) of chunk m at group m//GRP, col ((m%GRP)*3+k)*128."""
    s_base = b0 * W
    seg = ch_seg * W
    t = np.arange(s_base, s_base + seg)
    m_loc = (t - s_base) // W
    tl = t % W
    A = np.zeros((ch_seg, 4, W, W), np.float32)
    for j, g in ((0, g1p), (1, g2p), (2, g3p)):
        i = t - (zp[t] + 1 + j)
        c = t // W - i // W
        np.add.at(A, (m_loc, c, tl, i % W), g[t].astype(np.float32))
    A0 = A[:, 0]
    x_m = xp[s_base:s_base + seg].reshape(ch_seg, W).astype(np.float32)
    x_eff = x_m + np.einsum("mtu,mu->mt", A0, x_m)
    out = np.zeros((ngrp, KROW, GRP * 3 * KROW), BF16NP)
    for k, c in enumerate((3, 2, 1)):
        B = A[:, c] + np.matmul(A0, A[:, c])          # [m, tgt, src]
        Bt = np.ascontiguousarray(np.transpose(B, (0, 2, 1)))
        for m in range(ch_seg):
            g, off = divmod(m, GRP)
            col = (off * 3 + k) * KROW
            out[g, :W, col:col + W] = Bt[m].astype(BF16NP)
            if c == 3:
                out[g, W, col:col + W] = x_eff[m].astype(BF16NP)
    return out


# ------------------------------------------------------------- device kernel
def _build_nc(nb, ch_seg, ngrp):
    nc = bacc.Bacc(
        "TRN2", target_bir_lowering=False, debug=False, num_devices=N_CORES
    )
    wts = [
        nc.dram_tensor(f"wts{s}", [ngrp, KROW, GRP * 3 * KROW], BF16,
                       kind="ExternalInput")
        for s in range(SPC)
    ]
    init = nc.dram_tensor("init", [KROW, LEAD * nb], BF16,
                          kind="ExternalInput")
    ones = nc.dram_tensor("ones", [1, ch_seg * nb], BF16,
                          kind="ExternalInput")
    warm = nc.dram_tensor("warm", [KROW, 256], BF16, kind="ExternalInput")
    yout = nc.dram_tensor("yout", [SPC, ngrp, KROW, GRP * nb], BF16,
                          kind="ExternalOutput")
    with tile.TileContext(nc) as tc:
        with (
            tc.tile_pool(name="misc", bufs=1) as mpool,
            tc.tile_pool(name="ybuf", bufs=1) as ypool,
            tc.tile_pool(name="wpool", bufs=3) as wpool,
            tc.tile_pool(name="psum", bufs=8, space="PSUM") as ppool,
        ):
            wtile = mpool.tile([KROW, 256], BF16, tag="warm")
            nc.sync.dma_start(out=wtile[:, :], in_=warm[:, :])
            # ~5us of dummy matmuls: hold the PE busy through the HAM
            # activity window so real chunks start at 2.4 GHz
            wps = ppool.tile([W, 256], F32, tag="warmp", bufs=1)
            for i in range(9):
                nc.tensor.matmul(
                    wps[:, :], lhsT=wtile[:, 0:W], rhs=wtile[:, :],
                    start=True, stop=True,
                )
            # prime the ACT Copy table set off the critical path
            wact = mpool.tile([KROW, 256], BF16, tag="wact")
            nc.scalar.copy(wact[:, :], wtile[:, :])
            ylead = mpool.tile([KROW, LEAD * nb], BF16, tag="ylead")
            nc.sync.dma_start(out=ylead[:, :], in_=init[:, :])
            ytiles = [[None] * ngrp for _ in range(SPC)]

            def ycol(s, mm):
                if mm < 0:
                    c0 = (LEAD + mm) * nb
                    return ylead[:, c0:c0 + nb]
                g, off = divmod(mm, GRP)
                return ytiles[s][g][:, off * nb:(off + 1) * nb]

            # group 0 lands as interleaved 5-chunk pieces per slot so the
            # first pairs start ~4us earlier than two full-group waits
            wt = [None] * SPC
            wt0 = []
            c0 = 5 * 3 * KROW
            for s in range(SPC):
                t = wpool.tile(
                    [KROW, GRP * 3 * KROW], BF16,
                    name=f"w{s}", tag=f"w{s}", bufs=4,
                )
                nc.sync.dma_start(out=t[:, 0:c0], in_=wts[s][0][:, 0:c0])
                wt0.append(t)
            for s in range(SPC):
                nc.sync.dma_start(out=wt0[s][:, c0:], in_=wts[s][0][:, c0:])
            for m in range(ch_seg):
                for s in range(SPC):
                    g, off = divmod(m, GRP)
                    if off == 0:
                        if g == 0:
                            wt[s] = wt0[s]
                        else:
                            wt[s] = wpool.tile(
                                [KROW, GRP * 3 * KROW], BF16,
                                name=f"w{s}", tag=f"w{s}", bufs=4,
                            )
                            nc.sync.dma_start(
                                out=wt[s][:, :], in_=wts[s][g]
                            )
                        ytiles[s][g] = ypool.tile(
                            [KROW, GRP * nb], BF16,
                            name=f"y{s}g{g}", tag=f"y{s}g{g}",
                        )
                        nc.gpsimd.dma_start(
                            out=ytiles[s][g][W:KROW, :],
                            in_=ones[:, g * GRP * nb:(g + 1) * GRP * nb],
                        )
                    psum = ppool.tile([W, nb], F32, tag="acc", bufs=7)
                    for k, c in enumerate((3, 2, 1)):
                        col = (off * 3 + k) * KROW
                        nc.tensor.matmul(
                            psum[:, :],
                            lhsT=wt[s][:, col:col + W],
                            rhs=ycol(s, m - c),
                            start=(k == 0),
                            stop=(k == 2),
                        )
                    dst = ytiles[s][g][0:W, off * nb:(off + 1) * nb]
                    if s == 0:
                        nc.vector.tensor_copy(dst, psum[:, :])
                    else:
                        nc.scalar.copy(dst, psum[:, :])
                    if g == ngrp - 1 and off == 4:
                        # first half of the final group early: shortens
                        # the end-of-kernel store tail
                        nc.gpsimd.dma_start(
                            out=yout[s, g, :, 0:5 * nb],
                            in_=ytiles[s][g][:, 0:5 * nb],
                        )
                    if off == GRP - 1:
                        if g == ngrp - 1:
                            nc.gpsimd.dma_start(
                                out=yout[s, g, :, 5 * nb:],
                                in_=ytiles[s][g][:, 5 * nb:],
                            )
                        else:
                            nc.gpsimd.dma_start(
                                out=yout[s, g], in_=ytiles[s][g][:, :]
                            )
    nc.compile()
    return nc


_LAST_RESULT = {}


def kernel(delay_len_frames, raw_coeff_frames, excitation, n_samples):
    n = int(n_samples)
    z, g1, g2, g3, xfull = _host_preprocess(
        np.asarray(delay_len_frames), np.asarray(raw_coeff_frames),
        np.asarray(excitation), n,
    )
    assert n == N_SAMP, n
    nb, ch_seg, bounds = _choose_boundaries(z)
    ngrp = ch_seg // GRP
    lmax = nb - 1
    assert lmax <= LEAD * W
    assert int(z.min()) + 1 >= 64          # nilpotency of A_self

    npad = (NCH + ch_seg + 4) * W
    pad = npad - n
    zp = np.concatenate([z, np.full(pad, z[-1])]).astype(np.int64)
    g1p = np.concatenate([g1, np.full(pad, g1[-1])])
    g2p = np.concatenate([g2, np.full(pad, g2[-1])])
    g3p = np.concatenate([g3, np.full(pad, g3[-1])])
    xp = np.concatenate([xfull, np.zeros(pad)])

    init = np.zeros((KROW, LEAD * nb), BF16NP)
    for tt in range(LEAD):
        for r in range(W):
            j = r - (LEAD * W - lmax) + tt * W
            if 0 <= j < lmax:
                init[r, tt * nb + j] = BF16NP(1.0)
        init[W, tt * nb + nb - 1] = BF16NP(1.0)
    ones = np.zeros((1, ch_seg * nb), BF16NP)
    ones[0, nb - 1::nb] = BF16NP(1.0)
    warm = np.zeros((KROW, 256), BF16NP)

    in_maps = []
    for core in range(N_CORES):
        im = {"init": init, "ones": ones, "warm": warm}
        for s in range(SPC):
            k = core * SPC + s
            im[f"wts{s}"] = _build_slot_weights(
                bounds[k], ch_seg, ngrp, zp, g1p, g2p, g3p, xp
            )
        in_maps.append(im)

    nc = _build_nc(nb, ch_seg, ngrp)
    import os

    res = run_bass_kernel_spmd(
        nc,
        in_maps,
        core_ids=list(range(N_CORES)),
        trace=bool(os.environ.get("DIFFKS_TRACE")),
    )
    _LAST_RESULT["res"] = res

    y = np.zeros(NCH * W, np.float64)
    for k in range(NSEGT):
        core, s = divmod(k, SPC)
        H = res.results[core]["yout"][s][:, :W].astype(np.float32)
        # [ngrp, W, GRP*nb] -> [ch_seg*W, nb]
        H = H.reshape(ngrp, W, GRP, nb).transpose(0, 2, 1, 3)
        H = H.reshape(ch_seg * W, nb)
        s0 = bounds[k] * W
        ns = (bounds[k + 1] - bounds[k]) * W
        if k == 0:
            y_seg = H[:ns, lmax].astype(np.float64)
        else:
            s_k = y[s0 - lmax:s0]
            y_seg = H[:ns, :lmax].astype(np.float64) @ s_k + H[:ns, lmax]
        y[s0:s0 + ns] = y_seg
    return y[:n].astype(np.float32)
